# revision 1
# baseline (speedup 1.0000x reference)
"""BallQLoss kernel for 8 Trainium2 NeuronCores — windowed rewrite.

Computes mean_{b,i,k} |flow[b,i] - flow[b, idx[b,i,k]]|_1 where idx are the
first K=16 in-ball (radius 0.5) neighbors of each point in index order,
padded with the first neighbor (pointnet2 ball_query semantics).

Strategy: the host kd-partitions each batch's queries into 64 spatial tiles
of 128. For each tile it collects the candidate window — every point within
distance 0.5 of the tile's bounding box (a superset of every tile query's
in-ball set, by the triangle inequality) — sorts the window by original
index, and packs the windows back-to-back into per-core buffers. The device
then scores only window columns (fp32r matmul), builds descending-iota keys
over in-ball columns, takes the top-16 keys per query (first-16 by original
index), and gathers neighbor flows from a host-pre-permuted flow_win table.
Correct for any input by construction: the window provably contains every
in-ball point, so no prefix tuning, tail checks, or fallbacks are needed.

Per-slot pipeline:
  PE    : score = (r^2 - d^2)/2, augmented 5-row matmul in fp32r (1 cyc/col)
  ACT   : relu(score * 1e30) -> f16 {inf, 0}; later |nn - fq| via Abs+accum
  DVE   : keys = min(sgn, iota_desc); max8; zap via (keys < k8)*keys; max8
  Pool  : index arithmetic; one batched 16-slot indirect flow gather
"""

import numpy as np
from contextlib import ExitStack

K = 16
RADIUS = 0.5
B = 2
N = 8192
N_CORES = 8
CPB = N_CORES // B            # cores per batch
QPC = (B * N) // N_CORES      # 2048 queries per core
RT = 128                      # queries per tile (SBUF partition dim)
NT = QPC // RT                # 16 tiles per core
IOTA = 2048                   # iota table width (f16-exact integer range)
BLK = 512                     # PSUM bank width (fp32)

_cache: dict = {}


def _kd_split(q: np.ndarray) -> list[np.ndarray]:
    """Recursive equal-size median split on the widest dim -> 64 groups."""
    groups = [np.arange(len(q))]
    for _ in range(6):
        new = []
        for g in groups:
            pts = q[g]
            dim = int(np.argmax(pts.max(0) - pts.min(0)))
            order = g[np.argsort(pts[:, dim], kind="stable")]
            h = len(order) // 2
            new.append(order[:h])
            new.append(order[h:])
        groups = new
    return groups


def _plan(pc: np.ndarray) -> dict:
    """Host geometry: tiles, windows, per-core packing metadata."""
    r2 = np.float64(RADIUS * RADIUS)
    per_batch = []
    for b in range(B):
        q = pc[b].astype(np.float64)
        tiles = _kd_split(q)
        wins = []
        bounds = []
        eps = 1e-4  # margin vs device fp32r score rounding at the boundary
        for g in tiles:
            lo = q[g].min(0)
            hi = q[g].max(0)
            d = np.maximum(lo - q, 0.0) + np.maximum(q - hi, 0.0)
            cand = np.flatnonzero((d * d).sum(1) < r2)
            # exact window: points within RADIUS of at least one tile query
            dc = ((q[cand][:, None, :] - q[g][None, :, :]) ** 2).sum(-1)
            keep = (dc < r2).any(1)
            win = cand[keep]  # ascending original index
            dw = dc[keep]     # [W, 128] squared distances
            # Selection-range bounds. in_lo: definitely in-ball on device;
            # in_hi: possibly in-ball. The device's p-th in-ball position is
            # <= the in_lo-based one and >= the in_hi-based one.
            in_lo = dw < r2 - eps
            in_hi = dw < r2 + eps
            cs_lo = np.cumsum(in_lo, 0)
            cs_hi = np.cumsum(in_hi, 0)
            c_lo = cs_lo[-1]
            c_hi = cs_hi[-1]
            W = len(win)

            def kth_pos(cs, k):
                """1-based position of the k-th True per query (0 if k=0)."""
                return np.where(k > 0, (cs >= np.maximum(k, 1)).argmax(0) + 1,
                                0)

            # device in-ball set lies between in_lo and in_hi; bound the
            # position of its min(c,8)-th / min(c,16)-th point from above
            p8 = np.maximum(kth_pos(cs_lo, np.minimum(c_lo, 8)),
                            kth_pos(cs_hi, np.minimum(c_hi, 8)))
            E8 = int(p8.max())
            has9 = c_hi >= 9
            if has9.any():
                p8h = kth_pos(cs_hi, np.minimum(c_hi, 8))
                S2 = max(0, int(p8h[has9].min()) - 1)
                p16 = np.maximum(kth_pos(cs_lo, np.minimum(c_lo, 16)),
                                 kth_pos(cs_hi, np.minimum(c_hi, 16)))
                E16 = int(p16[has9].max())
                E16 = max(E16, min(W, S2 + 8))
            else:
                S2, E16 = 0, min(W, 8)
            # columns past max(E8, E16) can never be in any query's first-16
            Wt = max(E8, E16, 8)
            wins.append(win[:Wt])
            bounds.append((Wt, E8, S2, E16))
        # snake-deal width-sorted tiles to the batch's cores
        order = np.argsort([-len(w) for w in wins], kind="stable")
        core_tiles = [[] for _ in range(CPB)]
        for i, ti in enumerate(order):
            rnd, pos = divmod(i, CPB)
            c = pos if rnd % 2 == 0 else CPB - 1 - pos
            core_tiles[c].append(ti)
        # each core: sort its tiles by width descending, then arrange as a
        # pyramid (small, ..., large, ..., small) so the pipeline fills on a
        # small tile and drains on a small tile.
        for c in range(CPB):
            core_tiles[c].sort(key=lambda ti: -len(wins[ti]))
            desc = core_tiles[c]
            pyramid = [None] * NT
            mid = NT // 2
            for rank, p in enumerate(sorted(range(NT),
                                            key=lambda i: abs(i - mid))):
                pyramid[p] = desc[rank]
            core_tiles[c] = pyramid
        per_batch.append({"tiles": tiles, "wins": wins, "bounds": bounds,
                          "core_tiles": core_tiles})

    # slot geometry: aggregate across all 8 cores, quantized to 64
    def q64(x):
        return -(-x // 64) * 64

    wbars, e8s, s2s, e16s = [], [], [], []
    for s in range(NT):
        tb = [per_batch[b]["bounds"][per_batch[b]["core_tiles"][c][s]]
              for b in range(B) for c in range(CPB)]
        w = max(t[0] for t in tb)
        e8 = max(t[1] for t in tb)
        s2 = min(t[2] for t in tb)
        e16 = max(t[3] for t in tb)
        wq = min(IOTA, q64(w))
        assert w <= IOTA, f"window {w} exceeds iota range {IOTA}"
        wbars.append(wq)
        e8s.append(min(wq, q64(e8)))
        s2s.append((s2 // 64) * 64)
        e16s.append(min(wq, q64(e16)))
    coffs = np.concatenate([[0], np.cumsum(wbars)]).astype(np.int64)
    return {"per_batch": per_batch, "wbars": wbars, "e8s": e8s, "s2s": s2s,
            "e16s": e16s, "coffs": coffs}


def _build_program(wbars, e8s=None, s2s=None, e16s=None, repeat=1,
                   zap_pool=True, dve_key_slots=(0, NT - 1), mm_f32r=False,
                   gather_cols=1, debug=False):
    import concourse.bass as bass
    import concourse.tile as tile
    from concourse import bacc, bass_isa, mybir

    f32 = mybir.dt.float32
    f32r = mybir.dt.float32r
    f16 = mybir.dt.float16
    i32 = mybir.dt.int32
    u16 = mybir.dt.uint16
    i16 = mybir.dt.int16
    Alu = mybir.AluOpType
    Act = mybir.ActivationFunctionType

    wbars = list(wbars)
    if e8s is None:
        e8s = list(wbars)
    if s2s is None:
        s2s = [0] * NT
    if e16s is None:
        e16s = list(wbars)
    total_w = sum(wbars)
    coffs = [0]
    for w in wbars:
        coffs.append(coffs[-1] + w)

    nc = bacc.Bacc("TRN2", target_bir_lowering=False, debug=False,
                   num_devices=N_CORES)

    at = nc.dram_tensor("at", [5, QPC], f32, kind="ExternalInput").ap()
    btp = nc.dram_tensor("btp", [5, total_w], f32, kind="ExternalInput").ap()
    iota_in = nc.dram_tensor("iota_in", [1, IOTA], f16,
                             kind="ExternalInput").ap()
    # fw4: window flows as (x, y, z, 0) f16 rows, one table per slot,
    # broadcast to all 128 partitions for the gpsimd ap_gather.
    fw4d = nc.dram_tensor("fw4d", [1, total_w * 4], f16,
                          kind="ExternalInput").ap()
    # flowq: NEGATED query flows (x, y, z, 0) f16, slot-packed [RT, 4*NT].
    flowq = nc.dram_tensor("flowq", [RT, 4 * NT], f16,
                           kind="ExternalInput").ap()
    # m4: block-diagonal ownership mask [128, 256*4] f16.
    m4d = nc.dram_tensor("m4d", [RT, 256 * 4], f16,
                         kind="ExternalInput").ap()
    partial = nc.dram_tensor("partial", [RT, 1], f32,
                             kind="ExternalOutput").ap()
    if debug:
        cand_all = nc.dram_tensor("cand_all", [RT, NT * 16], f16,
                                  kind="ExternalOutput").ap()
        idx_all = nc.dram_tensor("idx_all", [RT, NT * 16], i32,
                                 kind="ExternalOutput").ap()
        nn_all = nc.dram_tensor("nn_all", [RT, NT * 48], f32,
                                kind="ExternalOutput").ap()

    with tile.TileContext(nc) as tc, ExitStack() as ctx:
        cpool = ctx.enter_context(tc.tile_pool(name="const", bufs=1))
        kpool = ctx.enter_context(tc.tile_pool(name="keys", bufs=4))
        ppool = ctx.enter_context(tc.tile_pool(name="ps", bufs=3, space="PSUM"))
        spool = ctx.enter_context(tc.tile_pool(name="small", bufs=6))

        # at/btp are loaded per-slot (below) so slot 0 starts immediately.
        at_sb = cpool.tile([5, QPC], f32)
        bt_sb = cpool.tile([5, total_w], f32)
        fq_sb = cpool.tile([RT, 4 * NT], f16)
        nc.scalar.dma_start(fq_sb[:], flowq[:])
        m4_sb = cpool.tile([RT, 256 * 4], f16)
        nc.scalar.dma_start(m4_sb[:], m4d[:])

        iota16 = cpool.tile([RT, IOTA], f16)
        nc.scalar.dma_start(iota16[:], iota_in[:].to_broadcast([RT, IOTA]))

        # PE warmup: an early throwaway matmul starts the ramp-to-full-clock
        # timer so the real matmuls run at a higher p-state. Operands are a
        # zeroed scratch tile; the result is never read.
        def mm_ap(ap):
            return ap.bitcast(f32r) if mm_f32r else ap

        wsrc = cpool.tile([5, 64], f32)
        nc.vector.memset(wsrc[:], 0.0)
        ones = cpool.tile([RT, 1], f32)
        nc.vector.memset(ones[:], 1.0)
        wpool = ctx.enter_context(tc.tile_pool(name="warm", bufs=1,
                                               space="PSUM"))
        warm = wpool.tile([64, 64], f32)
        nc.tensor.matmul(out=warm[:], lhsT=mm_ap(wsrc[:, 0:64]),
                         rhs=mm_ap(wsrc[:, 0:64]), start=True, stop=True)

        acc = cpool.tile([RT, NT], f32)

        rep_ctx = tc.For_i(0, repeat, 1) if repeat > 1 else None
        if rep_ctx is not None:
            rep_ctx.__enter__()

        for s in range(NT):
            W = wbars[s]
            C = coffs[s]
            nblk = (W + BLK - 1) // BLK

            if s % 4 == 0:
                nc.sync.dma_start(at_sb[:, s * RT:(s + 4) * RT],
                                  at[:, s * RT:(s + 4) * RT])
            nc.sync.dma_start(bt_sb[:, C:C + W], btp[:, C:C + W])

            # --- scores (1024-wide PSUM tiles) -> sgn -> keys.
            # TRN2 Pool/GPSIMD has no ALU tensor ops and cannot touch PSUM;
            # ACT does the relu, DVE the f16 min (2x mode).
            keys = kpool.tile([RT, W], f16, tag="keys")
            for c in range(0, W, 2 * BLK):
                cw = min(2 * BLK, W - c)
                ps = ppool.tile([RT, cw], f32, tag="ps")
                for g in range(0, cw, BLK):
                    w = min(BLK, cw - g)
                    nc.tensor.matmul(
                        out=ps[:, g:g + w],
                        lhsT=mm_ap(at_sb[:, s * RT:(s + 1) * RT]),
                        rhs=mm_ap(bt_sb[:, C + c + g:C + c + g + w]),
                        start=True, stop=True,
                    )
                sgn = kpool.tile([RT, cw], f16, tag="sgn")
                nc.scalar.activation(out=sgn[:], in_=ps[:],
                                     func=Act.Relu, scale=1e30)
                nc.vector.tensor_tensor(
                    out=keys[:, c:c + cw], in0=sgn[:],
                    in1=iota16[:, c:c + cw], op=Alu.min,
                )

            # --- top-16 keys = first-16 in-ball by original index.
            # First-8 live in [0, E8); ranks 9-16 in [S2, E16) (host bounds).
            # Zap mask via ACT: sign(t8 - keys) is +1 below the 8th-largest
            # key, 0 at it, -1 above; multiplying keeps ranks 9+ positive.
            E8, S2, E16 = e8s[s], s2s[s], e16s[s]
            Z = E16 - S2
            cand = spool.tile([RT, 16], f16, tag="cand")
            nc.vector.max(out=cand[:, 0:8], in_=keys[:, :E8])
            sgn2 = kpool.tile([RT, Z], f16, tag="sgn2")
            nc.scalar.activation(out=sgn2[:], in_=keys[:, S2:E16],
                                 func=Act.Sign, scale=-1.0,
                                 bias=cand[:, 7:8])
            keys2 = kpool.tile([RT, Z], f16, tag="keys2")
            nc.vector.tensor_tensor(out=keys2[:], in0=keys[:, S2:E16],
                                    in1=sgn2[:], op=Alu.mult)
            nc.vector.max(out=cand[:, 8:16], in_=keys2[:])

            # --- window-local ordinal: IOTA - key; pad empty slots (key <=
            # 0) with the first neighbor.
            valid = spool.tile([RT, 16], i32, tag="valid")
            nc.vector.tensor_scalar(valid[:], cand[:], 0.0, None, Alu.is_gt)
            idxf = spool.tile([RT, 16], f32, tag="idxf")
            nc.scalar.activation(out=idxf[:], in_=cand[:], func=Act.Copy,
                                 scale=-1.0, bias=float(IOTA))
            idxp = spool.tile([RT, 16], f32, tag="idxp")
            nc.gpsimd.tensor_copy(idxp[:],
                                  idxf[:, 0:1].to_broadcast([RT, 16]))
            nc.vector.copy_predicated(idxp[:], valid[:], idxf[:])
            idx = spool.tile([RT, 16], i16, tag="idx")
            nc.gpsimd.tensor_copy(idx[:], idxp[:])

            # --- gather neighbor flows via gpsimd ap_gather: each
            # 16-partition core gathers its 16 queries' 256 (q,k) window
            # rows from a per-partition-replicated fw4 table; the static
            # block-diagonal mask m4 keeps each channel's own 16.
            fw4 = spool.tile([RT, W * 4], f16, tag="fw4")
            bc_eng = nc.sync if s % 2 == 0 else nc.scalar
            bc_eng.dma_start(
                fw4[:], fw4d[:, C * 4:(C + W) * 4].to_broadcast([RT, W * 4]))
            g = spool.tile([RT, 256 * 4], f16, tag="g")
            nc.gpsimd.ap_gather(
                out_ap=g[:], in_ap=fw4[:], idxs_ap=idx[:],
                channels=RT, num_elems=W, d=4, num_idxs=256,
            )
            # dm = (g + (-fq4)) * m4, then ACT Abs + accum -> acc column.
            dm = spool.tile([RT, 256 * 4], f16, tag="dm")
            nc.vector.tensor_tensor(
                out=dm[:].rearrange("p (j d) -> p j d", d=4),
                in0=g[:].rearrange("p (j d) -> p j d", d=4),
                in1=fq_sb[:, 4 * s:4 * s + 4][:, None, :].to_broadcast(
                    [RT, 256, 4]),
                op=Alu.add)
            mm = spool.tile([RT, 256 * 4], f16, tag="mm")
            nc.vector.tensor_tensor(out=mm[:], in0=dm[:], in1=m4_sb[:],
                                    op=Alu.mult)
            difa = spool.tile([RT, 256 * 4], f16, tag="difa")
            nc.scalar.activation(
                out=difa[:], in_=mm[:], func=Act.Abs,
                accum_out=acc[:, s:s + 1],
            )
            if debug:
                nc.sync.dma_start(cand_all[:, s * 16:(s + 1) * 16], cand[:])
                nc.sync.dma_start(idx_all[:, s * 16:(s + 1) * 16], idx[:])
                nc.sync.dma_start(nn_all[:, s * 48:(s + 1) * 48], nn[:])

        if rep_ctx is not None:
            rep_ctx.__exit__(None, None, None)

        # --- final reduction: free axis on DVE; the 128-partition sum is
        # done on the host (it already sums the 8 per-core partials).
        accsum = cpool.tile([RT, 1], f32)
        nc.vector.tensor_reduce(out=accsum[:], in_=acc[:],
                                axis=mybir.AxisListType.X, op=Alu.add)
        nc.sync.dma_start(partial[:], accsum[:])

    nc.compile()
    return nc


def _in_maps(plan, pc, flow):
    wbars = plan["wbars"]
    coffs = plan["coffs"]
    total_w = int(coffs[-1])
    sq = (pc.astype(np.float64) ** 2).sum(axis=-1)
    r2 = np.float64(RADIUS * RADIUS)
    iota_in = (IOTA - np.arange(IOTA)).astype(np.float16).reshape(1, IOTA)

    maps = []
    for core in range(N_CORES):
        b = core // CPB
        c = core % CPB
        pb = plan["per_batch"][b]
        tids = pb["core_tiles"][c]

        perm = np.concatenate([pb["tiles"][t] for t in tids])
        q = pc[b, perm].astype(np.float64)
        at = np.concatenate(
            [q.T, sq[b, perm][None, :], np.ones((1, QPC))], axis=0
        ).astype(np.float32)

        # Padding columns score -1e4: safely negative, and -1e4 * 1e30 stays
        # finite in fp32 so the relu emits an exact 0 (no -inf -> NaN).
        btp = np.zeros((5, total_w), np.float32)
        btp[4, :] = np.float32(-1e4)
        fw4 = np.zeros((total_w, 4), np.float16)
        for s, t in enumerate(tids):
            win = pb["wins"][t]
            C = int(coffs[s])
            wlen = len(win)
            p = pc[b, win].astype(np.float64)
            btp[0:3, C:C + wlen] = p.T
            btp[3, C:C + wlen] = np.float32(-0.5)
            btp[4, C:C + wlen] = ((r2 - sq[b, win]) * 0.5)
            fw4[C:C + wlen, 0:3] = flow[b, win].astype(np.float16)
        # bt row 3 is the |q|^2 multiplier (-0.5); row 4 the constant term.
        fq4 = np.zeros((RT, 4 * NT), np.float16)
        fq3 = (-flow[b, perm]).reshape(NT, RT, 3).transpose(1, 0, 2)
        for s in range(NT):
            fq4[:, 4 * s:4 * s + 3] = fq3[:, s, :].astype(np.float16)
        ch = np.arange(RT) % 16
        j = np.arange(256) % 16
        m4 = (j[None, :] == ch[:, None]).astype(np.float16)
        m4 = np.repeat(m4[:, :, None], 4, axis=2).reshape(RT, 1024)
        maps.append({
            "at": np.ascontiguousarray(at),
            "btp": np.ascontiguousarray(btp),
            "iota_in": iota_in,
            "fw4d": np.ascontiguousarray(fw4.reshape(1, total_w * 4)),
            "flowq": np.ascontiguousarray(fq4),
            "m4d": np.ascontiguousarray(m4),
        })
    return maps


def kernel(pc: np.ndarray, flow: np.ndarray) -> np.ndarray:
    from concourse.bass_utils import run_bass_kernel_spmd

    pc = np.asarray(pc, dtype=np.float32)
    flow = np.asarray(flow, dtype=np.float32)

    plan = _plan(pc)
    key = tuple(plan["wbars"] + plan["e8s"] + plan["s2s"] + plan["e16s"])
    nc = _cache.get(key)
    if nc is None:
        nc = _build_program(plan["wbars"], plan["e8s"], plan["s2s"],
                            plan["e16s"])
        _cache[key] = nc

    maps = _in_maps(plan, pc, flow)
    res = run_bass_kernel_spmd(nc, maps, list(range(N_CORES)))

    total = np.float32(0.0)
    for core in range(N_CORES):
        total += res.results[core]["partial"].sum(dtype=np.float32)
    return np.float32(total / np.float32(B * N * K))



# revision 15
# speedup vs baseline: 6.7399x; 6.7399x over previous
"""BallQLoss kernel for 8 Trainium2 NeuronCores — windowed rewrite.

Computes mean_{b,i,k} |flow[b,i] - flow[b, idx[b,i,k]]|_1 where idx are the
first K=16 in-ball (radius 0.5) neighbors of each point in index order,
padded with the first neighbor (pointnet2 ball_query semantics).

Strategy: the host kd-partitions each batch's queries into 64 spatial tiles
of 128. For each tile it collects the candidate window — every point within
distance 0.5 of the tile's bounding box (a superset of every tile query's
in-ball set, by the triangle inequality) — sorts the window by original
index, and packs the windows back-to-back into per-core buffers. The device
then scores only window columns (fp32r matmul), builds descending-iota keys
over in-ball columns, takes the top-16 keys per query (first-16 by original
index), and gathers neighbor flows from a host-pre-permuted flow_win table.
Correct for any input by construction: the window provably contains every
in-ball point, so no prefix tuning, tail checks, or fallbacks are needed.

Per-slot pipeline:
  PE    : score = (r^2 - d^2)/2, augmented 5-row matmul in fp32r (1 cyc/col)
  ACT   : relu(score * 1e30) -> f16 {inf, 0}; later |nn - fq| via Abs+accum
  DVE   : keys = min(sgn, iota_desc); max8; zap via (keys < k8)*keys; max8
  Pool  : index arithmetic; one batched 16-slot indirect flow gather
"""

import numpy as np
from contextlib import ExitStack

K = 16
RADIUS = 0.5
B = 2
N = 8192
N_CORES = 8
CPB = N_CORES // B            # cores per batch
QPC = (B * N) // N_CORES      # 2048 queries per core
RT = 128                      # queries per tile (SBUF partition dim)
NT = QPC // RT                # 16 tiles per core
IOTA = 2048                   # iota table width (f16-exact integer range)
BLK = 512                     # PSUM bank width (fp32)

_cache: dict = {}


def _kd_split(q: np.ndarray) -> list[np.ndarray]:
    """Recursive equal-size median split on the widest dim -> 64 groups."""
    groups = [np.arange(len(q))]
    for _ in range(6):
        new = []
        for g in groups:
            pts = q[g]
            dim = int(np.argmax(pts.max(0) - pts.min(0)))
            order = g[np.argsort(pts[:, dim], kind="stable")]
            h = len(order) // 2
            new.append(order[:h])
            new.append(order[h:])
        groups = new
    return groups


def _plan(pc: np.ndarray) -> dict:
    """Host geometry: tiles, windows, per-core packing metadata."""
    r2 = np.float64(RADIUS * RADIUS)
    per_batch = []
    for b in range(B):
        q = pc[b].astype(np.float64)
        tiles = _kd_split(q)
        wins = []
        bounds = []
        eps = 1e-4  # margin vs device fp32r score rounding at the boundary
        for g in tiles:
            lo = q[g].min(0)
            hi = q[g].max(0)
            d = np.maximum(lo - q, 0.0) + np.maximum(q - hi, 0.0)
            cand = np.flatnonzero((d * d).sum(1) < r2)
            # exact window: points within RADIUS of at least one tile query
            dc = ((q[cand][:, None, :] - q[g][None, :, :]) ** 2).sum(-1)
            keep = (dc < r2).any(1)
            win = cand[keep]  # ascending original index
            dw = dc[keep]     # [W, 128] squared distances
            # Selection-range bounds. in_lo: definitely in-ball on device;
            # in_hi: possibly in-ball. The device's p-th in-ball position is
            # <= the in_lo-based one and >= the in_hi-based one.
            in_lo = dw < r2 - eps
            in_hi = dw < r2 + eps
            cs_lo = np.cumsum(in_lo, 0)
            cs_hi = np.cumsum(in_hi, 0)
            c_lo = cs_lo[-1]
            c_hi = cs_hi[-1]
            W = len(win)

            def kth_pos(cs, k):
                """1-based position of the k-th True per query (0 if k=0)."""
                return np.where(k > 0, (cs >= np.maximum(k, 1)).argmax(0) + 1,
                                0)

            # device in-ball set lies between in_lo and in_hi; bound the
            # position of its min(c,8)-th / min(c,16)-th point from above
            p8 = np.maximum(kth_pos(cs_lo, np.minimum(c_lo, 8)),
                            kth_pos(cs_hi, np.minimum(c_hi, 8)))
            E8 = int(p8.max())
            has9 = c_hi >= 9
            if has9.any():
                p8h = kth_pos(cs_hi, np.minimum(c_hi, 8))
                S2 = max(0, int(p8h[has9].min()) - 1)
                p16 = np.maximum(kth_pos(cs_lo, np.minimum(c_lo, 16)),
                                 kth_pos(cs_hi, np.minimum(c_hi, 16)))
                E16 = int(p16[has9].max())
                E16 = max(E16, min(W, S2 + 8))
            else:
                S2, E16 = 0, min(W, 8)
            # columns past max(E8, E16) can never be in any query's first-16
            Wt = max(E8, E16, 8)
            wins.append(win[:Wt])
            bounds.append((Wt, E8, S2, E16))
        # snake-deal width-sorted tiles to the batch's cores
        order = np.argsort([-len(w) for w in wins], kind="stable")
        core_tiles = [[] for _ in range(CPB)]
        for i, ti in enumerate(order):
            rnd, pos = divmod(i, CPB)
            c = pos if rnd % 2 == 0 else CPB - 1 - pos
            core_tiles[c].append(ti)
        # each core: sort its tiles by width descending, then arrange as a
        # pyramid (small, ..., large, ..., small) so the pipeline fills on a
        # small tile and drains on a small tile.
        for c in range(CPB):
            core_tiles[c].sort(key=lambda ti: -len(wins[ti]))
            desc = core_tiles[c]
            pyramid = [None] * NT
            mid = NT // 2
            for rank, p in enumerate(sorted(range(NT),
                                            key=lambda i: abs(i - mid))):
                pyramid[p] = desc[rank]
            core_tiles[c] = pyramid
        per_batch.append({"tiles": tiles, "wins": wins, "bounds": bounds,
                          "core_tiles": core_tiles})

    # slot geometry: aggregate across all 8 cores, quantized to 64
    def q64(x):
        return -(-x // 64) * 64

    wbars, e8s, s2s, e16s = [], [], [], []
    for s in range(NT):
        tb = [per_batch[b]["bounds"][per_batch[b]["core_tiles"][c][s]]
              for b in range(B) for c in range(CPB)]
        w = max(t[0] for t in tb)
        e8 = max(t[1] for t in tb)
        s2 = min(t[2] for t in tb)
        e16 = max(t[3] for t in tb)
        wq = min(IOTA, q64(w))
        assert w <= IOTA, f"window {w} exceeds iota range {IOTA}"
        wbars.append(wq)
        e8s.append(min(wq, q64(e8)))
        s2s.append((s2 // 64) * 64)
        e16s.append(min(wq, q64(e16)))
    coffs = np.concatenate([[0], np.cumsum(wbars)]).astype(np.int64)
    return {"per_batch": per_batch, "wbars": wbars, "e8s": e8s, "s2s": s2s,
            "e16s": e16s, "coffs": coffs}


def _build_program(wbars, e8s=None, s2s=None, e16s=None, repeat=1,
                   zap_pool=True, dve_key_slots=(0, NT - 1), mm_f32r=False,
                   gather_cols=1, debug=False, stages=99):
    import concourse.bass as bass
    import concourse.tile as tile
    from concourse import bacc, bass_isa, mybir

    f32 = mybir.dt.float32
    f32r = mybir.dt.float32r
    f16 = mybir.dt.float16
    i32 = mybir.dt.int32
    u16 = mybir.dt.uint16
    i16 = mybir.dt.int16
    Alu = mybir.AluOpType
    Act = mybir.ActivationFunctionType

    wbars = list(wbars)
    if e8s is None:
        e8s = list(wbars)
    if s2s is None:
        s2s = [0] * NT
    if e16s is None:
        e16s = list(wbars)
    total_w = sum(wbars)
    coffs = [0]
    for w in wbars:
        coffs.append(coffs[-1] + w)

    nc = bacc.Bacc("TRN2", target_bir_lowering=False, debug=False,
                   num_devices=N_CORES)

    at = nc.dram_tensor("at", [5, QPC], f32, kind="ExternalInput").ap()
    btp = nc.dram_tensor("btp", [5, total_w], f32, kind="ExternalInput").ap()
    iota_in = nc.dram_tensor("iota_in", [1, IOTA], f16,
                             kind="ExternalInput").ap()
    # fw4: window flows as (x, y, z, 0) f16 rows, one table per slot,
    # broadcast to all 128 partitions for the gpsimd ap_gather.
    fw4d = nc.dram_tensor("fw4d", [1, total_w * 4], f16,
                          kind="ExternalInput").ap()
    # flowq: NEGATED query flows (x, y, z, 0) f16, slot-packed [RT, 4*NT].
    flowq = nc.dram_tensor("flowq", [RT, 4 * NT], f16,
                           kind="ExternalInput").ap()
    # m4: block-diagonal ownership mask [128, 256*4] f16.
    m4d = nc.dram_tensor("m4d", [RT, 256 * 4], f16,
                         kind="ExternalInput").ap()
    partial = nc.dram_tensor("partial", [RT, 1], f32,
                             kind="ExternalOutput").ap()
    if debug:
        cand_all = nc.dram_tensor("cand_all", [RT, NT * 16], f16,
                                  kind="ExternalOutput").ap()
        idx_all = nc.dram_tensor("idx_all", [RT, NT * 16], i32,
                                 kind="ExternalOutput").ap()
        nn_all = nc.dram_tensor("nn_all", [RT, NT * 48], f32,
                                kind="ExternalOutput").ap()

    with tile.TileContext(nc) as tc, ExitStack() as ctx:
        cpool = ctx.enter_context(tc.tile_pool(name="const", bufs=1))
        kpool = ctx.enter_context(tc.tile_pool(name="keys", bufs=4))
        ppool = ctx.enter_context(tc.tile_pool(name="ps", bufs=3, space="PSUM"))
        spool = ctx.enter_context(tc.tile_pool(name="small", bufs=6))

        # at/btp are loaded per-slot (below) so slot 0 starts immediately.
        at_sb = cpool.tile([5, QPC], f32)
        bt_sb = cpool.tile([5, total_w], f32)
        fq_sb = cpool.tile([RT, 4 * NT], f16)
        nc.scalar.dma_start(fq_sb[:], flowq[:])
        m4_sb = cpool.tile([RT, 256 * 4], f16)
        nc.scalar.dma_start(m4_sb[:], m4d[:])

        iota16 = cpool.tile([RT, IOTA], f16)
        nc.scalar.dma_start(iota16[:], iota_in[:].to_broadcast([RT, IOTA]))

        # PE warmup: an early throwaway matmul starts the ramp-to-full-clock
        # timer so the real matmuls run at a higher p-state. Operands are a
        # zeroed scratch tile; the result is never read.
        def mm_ap(ap):
            return ap.bitcast(f32r) if mm_f32r else ap

        wsrc = cpool.tile([5, 64], f32)
        nc.vector.memset(wsrc[:], 0.0)
        ones = cpool.tile([RT, 1], f32)
        nc.vector.memset(ones[:], 1.0)
        wpool = ctx.enter_context(tc.tile_pool(name="warm", bufs=1,
                                               space="PSUM"))
        warm = wpool.tile([64, 64], f32)
        nc.tensor.matmul(out=warm[:], lhsT=mm_ap(wsrc[:, 0:64]),
                         rhs=mm_ap(wsrc[:, 0:64]), start=True, stop=True)

        acc = cpool.tile([RT, NT], f32)
        if stages < 5:
            nc.vector.memset(acc[:], 0.0)

        rep_ctx = tc.For_i(0, repeat, 1) if repeat > 1 else None
        if rep_ctx is not None:
            rep_ctx.__enter__()

        for s in range(NT):
            W = wbars[s]
            C = coffs[s]
            nblk = (W + BLK - 1) // BLK

            if s % 4 == 0:
                nc.sync.dma_start(at_sb[:, s * RT:(s + 4) * RT],
                                  at[:, s * RT:(s + 4) * RT])
            nc.sync.dma_start(bt_sb[:, C:C + W], btp[:, C:C + W])

            # --- scores (1024-wide PSUM tiles) -> sgn -> keys.
            # TRN2 Pool/GPSIMD has no ALU tensor ops and cannot touch PSUM;
            # ACT does the relu, DVE the f16 min (2x mode).
            keys = kpool.tile([RT, W], f16, tag="keys")
            for c in range(0, W, 2 * BLK):
                cw = min(2 * BLK, W - c)
                ps = ppool.tile([RT, cw], f32, tag="ps")
                for g in range(0, cw, BLK):
                    w = min(BLK, cw - g)
                    nc.tensor.matmul(
                        out=ps[:, g:g + w],
                        lhsT=mm_ap(at_sb[:, s * RT:(s + 1) * RT]),
                        rhs=mm_ap(bt_sb[:, C + c + g:C + c + g + w]),
                        start=True, stop=True,
                    )
                sgn = kpool.tile([RT, cw], f16, tag="sgn")
                nc.scalar.activation(out=sgn[:], in_=ps[:],
                                     func=Act.Relu, scale=1e30)
                nc.vector.tensor_tensor(
                    out=keys[:, c:c + cw], in0=sgn[:],
                    in1=iota16[:, c:c + cw], op=Alu.min,
                )

            if stages < 2:
                continue
            # --- top-16 keys = first-16 in-ball by original index.
            # First-8 live in [0, E8); ranks 9-16 in [S2, E16) (host bounds).
            # Zap mask via ACT: sign(t8 - keys) is +1 below the 8th-largest
            # key, 0 at it, -1 above; multiplying keeps ranks 9+ positive.
            E8, S2, E16 = e8s[s], s2s[s], e16s[s]
            Z = E16 - S2
            cand = spool.tile([RT, 16], f16, tag="cand")
            nc.vector.max(out=cand[:, 0:8], in_=keys[:, :E8])
            sgn2 = kpool.tile([RT, Z], f16, tag="sgn2")
            nc.scalar.activation(out=sgn2[:], in_=keys[:, S2:E16],
                                 func=Act.Sign, scale=-1.0,
                                 bias=cand[:, 7:8])
            keys2 = kpool.tile([RT, Z], f16, tag="keys2")
            nc.vector.tensor_tensor(out=keys2[:], in0=keys[:, S2:E16],
                                    in1=sgn2[:], op=Alu.mult)
            nc.vector.max(out=cand[:, 8:16], in_=keys2[:])

            if stages < 3:
                continue
            # --- window-local ordinal: IOTA - key; pad empty slots (key <=
            # 0) with the first neighbor.
            valid = spool.tile([RT, 16], i32, tag="valid")
            nc.vector.tensor_scalar(valid[:], cand[:], 0.0, None, Alu.is_gt)
            idxf = spool.tile([RT, 16], f32, tag="idxf")
            nc.scalar.activation(out=idxf[:], in_=cand[:], func=Act.Copy,
                                 scale=-1.0, bias=float(IOTA))
            idxp = spool.tile([RT, 16], f32, tag="idxp")
            nc.gpsimd.tensor_copy(idxp[:],
                                  idxf[:, 0:1].to_broadcast([RT, 16]))
            nc.vector.copy_predicated(idxp[:], valid[:], idxf[:])
            idx = spool.tile([RT, 16], i16, tag="idx")
            nc.gpsimd.tensor_copy(idx[:], idxp[:])

            if stages < 4:
                continue
            # --- gather neighbor flows via gpsimd ap_gather: each
            # 16-partition core gathers its 16 queries' 256 (q,k) window
            # rows from a per-partition-replicated fw4 table; the static
            # block-diagonal mask m4 keeps each channel's own 16.
            fw4 = spool.tile([RT, W * 4], f16, tag="fw4")
            bc_eng = nc.sync if s % 2 == 0 else nc.scalar
            bc_eng.dma_start(
                fw4[:], fw4d[:, C * 4:(C + W) * 4].to_broadcast([RT, W * 4]))
            if stages < 5:
                continue
            g = spool.tile([RT, 256 * 4], f16, tag="g")
            nc.gpsimd.ap_gather(
                out_ap=g[:], in_ap=fw4[:], idxs_ap=idx[:],
                channels=RT, num_elems=W, d=4, num_idxs=256,
            )
            if stages < 6:
                continue
            # dm = (g + (-fq4)) * m4, then ACT Abs + accum -> acc column.
            dm = spool.tile([RT, 256 * 4], f16, tag="dm")
            nc.vector.tensor_tensor(
                out=dm[:].rearrange("p (j d) -> p j d", d=4),
                in0=g[:].rearrange("p (j d) -> p j d", d=4),
                in1=fq_sb[:, 4 * s:4 * s + 4][:, None, :].to_broadcast(
                    [RT, 256, 4]),
                op=Alu.add)
            mm = spool.tile([RT, 256 * 4], f16, tag="mm")
            nc.vector.tensor_tensor(out=mm[:], in0=dm[:], in1=m4_sb[:],
                                    op=Alu.mult)
            difa = spool.tile([RT, 256 * 4], f16, tag="difa")
            nc.scalar.activation(
                out=difa[:], in_=mm[:], func=Act.Abs,
                accum_out=acc[:, s:s + 1],
            )
            if debug:
                nc.sync.dma_start(cand_all[:, s * 16:(s + 1) * 16], cand[:])
                nc.sync.dma_start(idx_all[:, s * 16:(s + 1) * 16], idx[:])
                nc.sync.dma_start(nn_all[:, s * 48:(s + 1) * 48], nn[:])

        if rep_ctx is not None:
            rep_ctx.__exit__(None, None, None)

        # --- final reduction: free axis on DVE; the 128-partition sum is
        # done on the host (it already sums the 8 per-core partials).
        accsum = cpool.tile([RT, 1], f32)
        nc.vector.tensor_reduce(out=accsum[:], in_=acc[:],
                                axis=mybir.AxisListType.X, op=Alu.add)
        nc.sync.dma_start(partial[:], accsum[:])

    nc.compile()
    return nc


def _build_v3(wbars, repeat=1):
    """Gather-free pipeline.

    Per slot: fp32 matmul scores -> b = (score > 0) -> r = cumsum(b) via
    tensor_tensor_scan -> multiplicity w = b*(r<=16) + pad*b*(r==1) with
    pad = relu(16 - count) -> |flow_q - flow_win| per dim via ACT Abs with
    per-partition bias -> chained tensor_tensor_reduce sum(w * D).
    """
    import concourse.tile as tile
    from concourse import bacc, mybir

    f32 = mybir.dt.float32
    f16 = mybir.dt.float16
    Alu = mybir.AluOpType
    Act = mybir.ActivationFunctionType

    wbars = list(wbars)
    total_w = sum(wbars)
    coffs = [0]
    for w in wbars:
        coffs.append(coffs[-1] + w)

    nc = bacc.Bacc("TRN2", target_bir_lowering=False, debug=False,
                   num_devices=N_CORES)

    at = nc.dram_tensor("at", [5, QPC], f32, kind="ExternalInput").ap()
    btp = nc.dram_tensor("btp", [5, total_w], f32, kind="ExternalInput").ap()
    fw3d = nc.dram_tensor("fw3d", [1, 3 * total_w], f16,
                          kind="ExternalInput").ap()
    fq3d = nc.dram_tensor("fq3d", [RT, 3 * NT], f32,
                          kind="ExternalInput").ap()
    partial = nc.dram_tensor("partial", [RT, 1], f32,
                             kind="ExternalOutput").ap()

    GRP = 4                       # slots per TTR reduction group
    NG = NT // GRP

    with tile.TileContext(nc) as tc, ExitStack() as ctx:
        cpool = ctx.enter_context(tc.tile_pool(name="const", bufs=1))
        ipool = ctx.enter_context(tc.tile_pool(name="iter", bufs=2))
        ppool = ctx.enter_context(tc.tile_pool(name="ps", bufs=1,
                                               space="PSUM"))

        at_sb = cpool.tile([5, QPC], f32)
        bt_sb = cpool.tile([5, total_w], f32)
        fw_sb = cpool.tile([RT, 3 * total_w], f16)
        nc.scalar.dma_start(fw_sb[:], fw3d[:].to_broadcast(
            [RT, 3 * total_w]))
        fq_sb = cpool.tile([RT, 3 * NT], f32)
        nc.scalar.dma_start(fq_sb[:], fq3d[:])

        # PE warmup: early throwaway matmul starts the clock-ramp timer.
        wsrc = cpool.tile([5, 64], f32)
        nc.vector.memset(wsrc[:], 0.0)
        wpool = ctx.enter_context(tc.tile_pool(name="warm", bufs=1,
                                               space="PSUM"))
        warm = wpool.tile([64, 64], f32)
        nc.tensor.matmul(out=warm[:], lhsT=wsrc[:, 0:64],
                         rhs=wsrc[:, 0:64], start=True, stop=True)

        bias16 = cpool.tile([RT, 1], f32)
        nc.vector.memset(bias16[:], 16.0)

        rep_ctx = tc.For_i(0, repeat, 1) if repeat > 1 else None
        if rep_ctx is not None:
            rep_ctx.__enter__()

        ps = ppool.tile([RT, total_w], f32, tag="ps")
        b_t = ipool.tile([RT, total_w], f16, tag="b_t")
        r_t = ipool.tile([RT, total_w], f16, tag="r_t")
        m_t = ipool.tile([RT, total_w], f16, tag="m_t")
        e_t = ipool.tile([RT, total_w], f16, tag="e_t")
        w_t = ipool.tile([RT, total_w], f16, tag="w_t")
        dx_t = ipool.tile([RT, total_w], f16, tag="dx_t")
        dy_t = ipool.tile([RT, total_w], f16, tag="dy_t")
        dz_t = ipool.tile([RT, total_w], f16, tag="dz_t")
        junk = ipool.tile([RT, total_w], f16, tag="junk")
        pad = ipool.tile([RT, NT], f32, tag="pad")
        acc = ipool.tile([RT, 3 * NG], f32, tag="acc")

        dts = [dx_t, dy_t, dz_t]
        nacc = 0
        for s in range(NT):
            W = wbars[s]
            C = coffs[s]

            if s % 4 == 0:
                nc.sync.dma_start(at_sb[:, s * RT:(s + 4) * RT],
                                  at[:, s * RT:(s + 4) * RT])
            nc.sync.dma_start(bt_sb[:, C:C + W], btp[:, C:C + W])

            g = C
            while g < C + W:
                # stay within one 512-wide PSUM bank per matmul
                g1 = min(C + W, (g // BLK + 1) * BLK)
                nc.tensor.matmul(
                    out=ps[:, g:g1],
                    lhsT=at_sb[:, s * RT:(s + 1) * RT],
                    rhs=bt_sb[:, g:g1],
                    start=True, stop=True,
                )
                g = g1

            nc.vector.tensor_scalar(b_t[:, C:C + W], ps[:, C:C + W],
                                    0.0, None, Alu.is_gt)
            # r = inclusive cumsum of b (per-slot scan; initial 0; op1
            # bypass ignores data1)
            nc.vector.tensor_tensor_scan(
                out=r_t[:, C:C + W], data0=b_t[:, C:C + W],
                data1=b_t[:, C:C + W], initial=0.0,
                op0=Alu.add, op1=Alu.bypass,
            )
            # m = b * (r <= 16): first-16 in-ball multiplicity base
            nc.vector.scalar_tensor_tensor(
                out=m_t[:, C:C + W], in0=r_t[:, C:C + W], scalar=16.0,
                in1=b_t[:, C:C + W], op0=Alu.is_le, op1=Alu.mult)
            # e = b * (r == 1): one-hot of the first in-ball column
            nc.vector.scalar_tensor_tensor(
                out=e_t[:, C:C + W], in0=r_t[:, C:C + W], scalar=1.0,
                in1=b_t[:, C:C + W], op0=Alu.is_equal, op1=Alu.mult)
            # pad = relu(16 - count); count = r at the slot's last column
            nc.scalar.activation(out=pad[:, s:s + 1],
                                 in_=r_t[:, C + W - 1:C + W],
                                 func=Act.Relu, scale=-1.0, bias=bias16[:])
            # w = e * pad + m
            nc.vector.scalar_tensor_tensor(
                out=w_t[:, C:C + W], in0=e_t[:, C:C + W],
                scalar=pad[:, s:s + 1], in1=m_t[:, C:C + W],
                op0=Alu.mult, op1=Alu.add)
            # D_d = |flow_win_d - flow_q_d| via Abs(-x + bias)
            for d in range(3):
                nc.scalar.activation(
                    out=dts[d][:, C:C + W],
                    in_=fw_sb[:, d * total_w + C:d * total_w + C + W],
                    func=Act.Abs, scale=-1.0,
                    bias=fq_sb[:, d * NT + s:d * NT + s + 1])

            if s % GRP == GRP - 1:
                G0 = coffs[s + 1 - GRP]
                G1 = coffs[s + 1]
                for d in range(3):
                    nc.vector.scalar_tensor_tensor(
                        out=junk[:, G0:G1], in0=dts[d][:, G0:G1],
                        scalar=1.0, in1=w_t[:, G0:G1],
                        op0=Alu.mult, op1=Alu.mult,
                        accum_out=acc[:, nacc:nacc + 1])
                    nacc += 1

        accsum = ipool.tile([RT, 1], f32, tag="accsum")
        nc.vector.tensor_reduce(out=accsum[:], in_=acc[:],
                                axis=mybir.AxisListType.X, op=Alu.add)

        if rep_ctx is not None:
            rep_ctx.__exit__(None, None, None)

        nc.sync.dma_start(partial[:], accsum[:])

    nc.compile()
    return nc


def _in_maps_v3(plan, pc, flow):
    wbars = plan["wbars"]
    coffs = plan["coffs"]
    total_w = int(coffs[-1])
    sq = (pc.astype(np.float64) ** 2).sum(axis=-1)
    r2 = np.float64(RADIUS * RADIUS)

    maps = []
    for core in range(N_CORES):
        b = core // CPB
        c = core % CPB
        pb = plan["per_batch"][b]
        tids = pb["core_tiles"][c]

        perm = np.concatenate([pb["tiles"][t] for t in tids])
        q = pc[b, perm].astype(np.float64)
        at = np.concatenate(
            [q.T, sq[b, perm][None, :], np.ones((1, QPC))], axis=0
        ).astype(np.float32)

        btp = np.zeros((5, total_w), np.float32)
        btp[4, :] = np.float32(-1e4)
        fw3 = np.zeros((1, 3 * total_w), np.float16)
        for s, t in enumerate(tids):
            win = pb["wins"][t]
            C = int(coffs[s])
            wlen = len(win)
            p = pc[b, win].astype(np.float64)
            btp[0:3, C:C + wlen] = p.T
            btp[3, C:C + wlen] = np.float32(-0.5)
            btp[4, C:C + wlen] = ((r2 - sq[b, win]) * 0.5)
            for d in range(3):
                fw3[0, d * total_w + C:d * total_w + C + wlen] = \
                    flow[b, win, d].astype(np.float16)
        fq3 = np.zeros((RT, 3 * NT), np.float32)
        fq = flow[b, perm].reshape(NT, RT, 3)
        for s in range(NT):
            for d in range(3):
                fq3[:, d * NT + s] = fq[s, :, d]
        maps.append({
            "at": np.ascontiguousarray(at),
            "btp": np.ascontiguousarray(btp),
            "fw3d": np.ascontiguousarray(fw3),
            "fq3d": np.ascontiguousarray(fq3),
        })
    return maps


def _in_maps(plan, pc, flow):
    wbars = plan["wbars"]
    coffs = plan["coffs"]
    total_w = int(coffs[-1])
    sq = (pc.astype(np.float64) ** 2).sum(axis=-1)
    r2 = np.float64(RADIUS * RADIUS)
    iota_in = (IOTA - np.arange(IOTA)).astype(np.float16).reshape(1, IOTA)

    maps = []
    for core in range(N_CORES):
        b = core // CPB
        c = core % CPB
        pb = plan["per_batch"][b]
        tids = pb["core_tiles"][c]

        perm = np.concatenate([pb["tiles"][t] for t in tids])
        q = pc[b, perm].astype(np.float64)
        at = np.concatenate(
            [q.T, sq[b, perm][None, :], np.ones((1, QPC))], axis=0
        ).astype(np.float32)

        # Padding columns score -1e4: safely negative, and -1e4 * 1e30 stays
        # finite in fp32 so the relu emits an exact 0 (no -inf -> NaN).
        btp = np.zeros((5, total_w), np.float32)
        btp[4, :] = np.float32(-1e4)
        fw4 = np.zeros((total_w, 4), np.float16)
        for s, t in enumerate(tids):
            win = pb["wins"][t]
            C = int(coffs[s])
            wlen = len(win)
            p = pc[b, win].astype(np.float64)
            btp[0:3, C:C + wlen] = p.T
            btp[3, C:C + wlen] = np.float32(-0.5)
            btp[4, C:C + wlen] = ((r2 - sq[b, win]) * 0.5)
            fw4[C:C + wlen, 0:3] = flow[b, win].astype(np.float16)
        # bt row 3 is the |q|^2 multiplier (-0.5); row 4 the constant term.
        fq4 = np.zeros((RT, 4 * NT), np.float16)
        fq3 = (-flow[b, perm]).reshape(NT, RT, 3).transpose(1, 0, 2)
        for s in range(NT):
            fq4[:, 4 * s:4 * s + 3] = fq3[:, s, :].astype(np.float16)
        ch = np.arange(RT) % 16
        j = np.arange(256) % 16
        m4 = (j[None, :] == ch[:, None]).astype(np.float16)
        m4 = np.repeat(m4[:, :, None], 4, axis=2).reshape(RT, 1024)
        maps.append({
            "at": np.ascontiguousarray(at),
            "btp": np.ascontiguousarray(btp),
            "iota_in": iota_in,
            "fw4d": np.ascontiguousarray(fw4.reshape(1, total_w * 4)),
            "flowq": np.ascontiguousarray(fq4),
            "m4d": np.ascontiguousarray(m4),
        })
    return maps


def kernel(pc: np.ndarray, flow: np.ndarray) -> np.ndarray:
    from concourse.bass_utils import run_bass_kernel_spmd

    pc = np.asarray(pc, dtype=np.float32)
    flow = np.asarray(flow, dtype=np.float32)

    plan = _plan(pc)
    key = tuple(plan["wbars"])
    nc = _cache.get(key)
    if nc is None:
        nc = _build_v3(plan["wbars"])
        _cache[key] = nc

    maps = _in_maps_v3(plan, pc, flow)
    res = run_bass_kernel_spmd(nc, maps, list(range(N_CORES)))

    total = np.float32(0.0)
    for core in range(N_CORES):
        total += res.results[core]["partial"].sum(dtype=np.float32)
    return np.float32(total / np.float32(B * N * K))

